# revision 3
# baseline (speedup 1.0000x reference)
"""Bayesian linear layer on 8 TRN2 NeuronCores.

Computes  out = x @ (mu + softplus(rho) * eps_w).T + (bmu + softplus(brho) * eps_b)
for x [16384, 4096], weights [4096, 4096].

Sharding: 2-way split of the batch dim (N) x 4-way split of out_features.
Each core computes an [8192, 1024] fp32 output shard:
  - weight shard W^T is generated on-device (softplus via Exp+Ln(x+1), FMA on
    DVE, bf16 output, xbar DMA-transpose into [in_f, out_f] layout) and kept
    SBUF-resident as 32 tiles [128, 1024] bf16.
  - x is shipped bf16 and loaded transposed straight from DRAM via the xbar
    (2-byte DMA transpose), in panels of [4096, NB].
  - matmuls are bf16, N=512, accumulating fp32 in PSUM over the 32 k-blocks;
    bias (also generated on device) is added during the PSUM->SBUF copy.
"""

import numpy as np
import ml_dtypes

import concourse.bacc as bacc
import concourse.tile as tile
from concourse import mybir
from concourse import bass_utils

R, C = 2, 4                      # grid: R-way split of N, C-way split of out_f
N, IN_F, OUT_F = 16384, 4096, 4096
NS, OS = N // R, OUT_F // C      # per-core shards: 8192 rows, 1024 out cols
KB = IN_F // 128                 # 32 k-blocks
NB = 256                         # x^T panel width (n rows per super-tile)
N_CORES = 8

FP32 = mybir.dt.float32
BF16 = mybir.dt.float16


def _build_nc():
    nc = bacc.Bacc("TRN2", target_bir_lowering=False, debug=False)

    xb = nc.dram_tensor("xb", [NS, IN_F], BF16, kind="ExternalInput").ap()
    mu = nc.dram_tensor("mu", [OS, IN_F], FP32, kind="ExternalInput").ap()
    rho = nc.dram_tensor("rho", [OS, IN_F], FP32, kind="ExternalInput").ap()
    eps = nc.dram_tensor("eps", [OS, IN_F], FP32, kind="ExternalInput").ap()
    bmu = nc.dram_tensor("bmu", [128, OS], FP32, kind="ExternalInput").ap()
    brho = nc.dram_tensor("brho", [128, OS], FP32, kind="ExternalInput").ap()
    beps = nc.dram_tensor("beps", [128, OS], FP32, kind="ExternalInput").ap()
    out = nc.dram_tensor("out", [NS, OS], FP32, kind="ExternalOutput").ap()

    AF = mybir.ActivationFunctionType

    with tile.TileContext(nc) as tc:
        with (
            tc.tile_pool(name="wt", bufs=1) as wt_pool,
            tc.tile_pool(name="bias", bufs=1) as bias_pool,
            tc.tile_pool(name="prep_in", bufs=2) as prep_in,
            tc.tile_pool(name="prep_tmp", bufs=2) as prep_tmp,
            tc.tile_pool(name="xt", bufs=2) as xt_pool,
            tc.tile_pool(name="outp", bufs=3) as out_pool,
            tc.tile_pool(name="psum", bufs=4, space="PSUM") as psum_pool,
        ):
            # ---- bias: b = bmu + softplus(brho) * beps, replicated [128, OS]
            bmu_t = bias_pool.tile([128, OS], FP32, tag="bmu")
            brho_t = bias_pool.tile([128, OS], FP32, tag="brho")
            beps_t = bias_pool.tile([128, OS], FP32, tag="beps")
            nc.sync.dma_start(bmu_t[:], bmu[:])
            nc.sync.dma_start(brho_t[:], brho[:])
            nc.sync.dma_start(beps_t[:], beps[:])
            bex = bias_pool.tile([128, OS], FP32, tag="bex")
            nc.scalar.activation(bex[:], brho_t[:], AF.Exp)
            bsp = bias_pool.tile([128, OS], FP32, tag="bsp")
            nc.scalar.activation(bsp[:], bex[:], AF.Ln, bias=1.0)
            bt0 = bias_pool.tile([128, OS], FP32, tag="bt0")
            nc.vector.tensor_mul(bt0[:], bsp[:], beps_t[:])
            bias_t = bias_pool.tile([128, OS], FP32, tag="bias")
            nc.vector.tensor_add(bias_t[:], bt0[:], bmu_t[:])

            # ---- W^T prep: 32 persistent tiles [128 (i), OS (o)] bf16
            wts = [wt_pool.tile([128, OS], BF16, tag=f"wt{ib}", name=f"wt{ib}")
                   for ib in range(KB)]

            IC = 1024  # i-chunk width for the elementwise pass
            for ob in range(OS // 128):          # 8 o-row blocks
                for ic in range(IN_F // IC):     # 4 i chunks
                    mu_c = prep_in.tile([128, IC], FP32, tag="mu")
                    rho_c = prep_in.tile([128, IC], FP32, tag="rho")
                    eps_c = prep_in.tile([128, IC], FP32, tag="eps")
                    sl = (slice(ob * 128, (ob + 1) * 128),
                          slice(ic * IC, (ic + 1) * IC))
                    nc.sync.dma_start(mu_c[:], mu[sl])
                    nc.sync.dma_start(rho_c[:], rho[sl])
                    nc.sync.dma_start(eps_c[:], eps[sl])
                    ex = prep_tmp.tile([128, IC], FP32, tag="ex")
                    nc.scalar.activation(ex[:], rho_c[:], AF.Exp)
                    sp = prep_tmp.tile([128, IC], FP32, tag="sp")
                    nc.scalar.activation(sp[:], ex[:], AF.Ln, bias=1.0)
                    t0 = prep_tmp.tile([128, IC], FP32, tag="t0")
                    nc.vector.tensor_mul(t0[:], sp[:], eps_c[:])
                    wbf = prep_tmp.tile([128, IC], BF16, tag="wbf")
                    nc.vector.tensor_add(wbf[:], t0[:], mu_c[:])
                    # transpose each [128 o, 128 i] block into WT[ib][:, ob*128:...]
                    for j in range(IC // 128):
                        ib = ic * (IC // 128) + j
                        nc.sync.dma_start(
                            wts[ib][:, ob * 128:(ob + 1) * 128],
                            wbf[:, j * 128:(j + 1) * 128],
                            transpose=True,
                        )

            # ---- main loop: super-tiles of NB rows of x
            n_super = NS // NB
            subs = NB // 128
            for s in range(n_super):
                xt = xt_pool.tile([128, KB * NB], BF16)
                for ib in range(KB):
                    nc.sync.dma_start(
                        xt[:, ib * NB:(ib + 1) * NB],
                        xb[s * NB:(s + 1) * NB, ib * 128:(ib + 1) * 128],
                        transpose=True,
                    )
                for sub in range(subs):
                    ps0 = psum_pool.tile([128, 512], FP32)
                    ps1 = psum_pool.tile([128, 512], FP32)
                    for ib in range(KB):
                        xs = xt[:, ib * NB + sub * 128: ib * NB + (sub + 1) * 128]
                        nc.tensor.matmul(ps0[:], xs, wts[ib][:, 0:512],
                                         start=(ib == 0), stop=(ib == KB - 1))
                        nc.tensor.matmul(ps1[:], xs, wts[ib][:, 512:1024],
                                         start=(ib == 0), stop=(ib == KB - 1))
                    ob_t = out_pool.tile([128, OS], FP32)
                    nc.vector.tensor_add(ob_t[:, 0:512], ps0[:], bias_t[:, 0:512])
                    nc.vector.tensor_add(ob_t[:, 512:1024], ps1[:], bias_t[:, 512:1024])
                    row = (s * subs + sub) * 128
                    nc.sync.dma_start(out[row:row + 128, :], ob_t[:])

    nc.compile()
    return nc


_NC = None


def _get_nc():
    global _NC
    if _NC is None:
        _NC = _build_nc()
    return _NC


def kernel(x, weight_mu, weight_rho, bias_mu, bias_rho, eps_w, eps_b,
           _trace=False, _trace_kwargs=None):
    x = np.asarray(x, dtype=np.float32)
    weight_mu = np.asarray(weight_mu, dtype=np.float32)
    weight_rho = np.asarray(weight_rho, dtype=np.float32)
    bias_mu = np.asarray(bias_mu, dtype=np.float32)
    bias_rho = np.asarray(bias_rho, dtype=np.float32)
    eps_w = np.asarray(eps_w, dtype=np.float32)
    eps_b = np.asarray(eps_b, dtype=np.float32)

    nc = _get_nc()
    xb = x.astype(np.float16)

    in_maps = []
    for c in range(N_CORES):
        r, q = divmod(c, C)
        osl = slice(q * OS, (q + 1) * OS)
        in_maps.append({
            "xb": xb[r * NS:(r + 1) * NS],
            "mu": weight_mu[osl],
            "rho": weight_rho[osl],
            "eps": eps_w[osl],
            "bmu": np.ascontiguousarray(np.broadcast_to(bias_mu[osl], (128, OS))),
            "brho": np.ascontiguousarray(np.broadcast_to(bias_rho[osl], (128, OS))),
            "beps": np.ascontiguousarray(np.broadcast_to(eps_b[osl], (128, OS))),
        })

    kwargs = {}
    if _trace:
        kwargs["trace"] = True
        if _trace_kwargs:
            kwargs.update(_trace_kwargs)
    res = bass_utils.run_bass_kernel_spmd(
        nc, in_maps, core_ids=list(range(N_CORES)), **kwargs)

    out = np.empty((N, OUT_F), np.float32)
    for c in range(N_CORES):
        r, q = divmod(c, C)
        out[r * NS:(r + 1) * NS, q * OS:(q + 1) * OS] = res.results[c]["out"]
    if _trace:
        return out, res
    return out


# revision 5
# speedup vs baseline: 1.3806x; 1.3806x over previous
"""Bayesian linear layer on 8 TRN2 NeuronCores.

Computes  out = x @ (mu + softplus(rho) * eps_w).T + (bmu + softplus(brho) * eps_b)
for x [16384, 4096], weights [4096, 4096].

Sharding: 2-way split of the batch dim (N) x 4-way split of out_features.
Each core computes an [8192, 1024] fp32 output shard:
  - weight shard W^T is generated on-device (softplus via Exp + Ln(x+1), FMA
    on DVE, fp16 output, xbar DMA-transpose into [in_f, out_f] layout) and
    kept SBUF-resident as 64 tiles [128, 512] fp16.
  - x is shipped fp16 and loaded transposed straight from DRAM via the xbar
    (2-byte DMA transpose) in panels of [4096, NB]; transpose DMAs alternate
    between the SP and ACT HWDGE rings (each ring costs ~1.3us/op serially).
  - matmuls are fp16, N=512, accumulating fp32 in PSUM over the 32 k-blocks;
    bias (generated on device from host-replicated [128, OS] inputs) is added
    during the PSUM->SBUF copy on DVE.
"""

import numpy as np

import concourse.bacc as bacc
import concourse.tile as tile
from concourse import mybir
from concourse import bass_utils

R, C = 2, 4                      # grid: R-way split of N, C-way split of out_f
N, IN_F, OUT_F = 16384, 4096, 4096
NS, OS = N // R, OUT_F // C      # per-core shards: 8192 rows, 1024 out cols
KB = IN_F // 128                 # 32 k-blocks
NB = 512                         # x^T panel width (n rows per super-tile)
N_CORES = 8

FP32 = mybir.dt.float32
F16 = mybir.dt.float16


def _build_nc():
    nc = bacc.Bacc("TRN2", target_bir_lowering=False, debug=False)

    xb = nc.dram_tensor("xb", [NS, IN_F], F16, kind="ExternalInput").ap()
    mu = nc.dram_tensor("mu", [OS, IN_F], FP32, kind="ExternalInput").ap()
    rho = nc.dram_tensor("rho", [OS, IN_F], FP32, kind="ExternalInput").ap()
    eps = nc.dram_tensor("eps", [OS, IN_F], FP32, kind="ExternalInput").ap()
    bmu = nc.dram_tensor("bmu", [128, OS], FP32, kind="ExternalInput").ap()
    brho = nc.dram_tensor("brho", [128, OS], FP32, kind="ExternalInput").ap()
    beps = nc.dram_tensor("beps", [128, OS], FP32, kind="ExternalInput").ap()
    out = nc.dram_tensor("out", [NS, OS], FP32, kind="ExternalOutput").ap()

    AF = mybir.ActivationFunctionType
    import os as _os
    _split = _os.environ.get("K_RING_SPLIT", "1") == "1"
    rings = [nc.sync, nc.scalar if _split else nc.sync]

    with tile.TileContext(nc) as tc:
        with (
            tc.tile_pool(name="wt", bufs=1) as wt_pool,
            tc.tile_pool(name="bias", bufs=1) as bias_pool,
            tc.tile_pool(name="prep_rho", bufs=1) as prep_rho,
            tc.tile_pool(name="prep_in", bufs=2) as prep_in,
            tc.tile_pool(name="prep_w", bufs=2) as prep_w,
            tc.tile_pool(name="xt", bufs=2) as xt_pool,
            tc.tile_pool(name="outp", bufs=3) as out_pool,
            tc.tile_pool(name="psum", bufs=4, space="PSUM") as psum_pool,
        ):
            # ---- bias: b = bmu + softplus(brho) * beps, replicated [128, OS]
            bmu_t = bias_pool.tile([128, OS], FP32, tag="bmu")
            brho_t = bias_pool.tile([128, OS], FP32, tag="brho")
            beps_t = bias_pool.tile([128, OS], FP32, tag="beps")
            nc.sync.dma_start(bmu_t[:], bmu[:])
            nc.sync.dma_start(brho_t[:], brho[:])
            nc.sync.dma_start(beps_t[:], beps[:])
            # softplus in place over brho_t (Exp, then Ln(x+1))
            nc.scalar.activation(brho_t[:], brho_t[:], AF.Exp)
            nc.scalar.activation(brho_t[:], brho_t[:], AF.Ln, bias=1.0)
            nc.vector.tensor_mul(beps_t[:], brho_t[:], beps_t[:])
            bias_t = bias_pool.tile([128, OS], FP32, tag="bias")
            nc.vector.tensor_add(bias_t[:], beps_t[:], bmu_t[:])

            # ---- W^T prep: 64 persistent tiles [128 (i), 512 (o)] fp16,
            # indexed [ib][q].  Fill q=0 (ob 0..3) first so matmuls can start.
            wts = [[wt_pool.tile([128, 512], F16, tag=f"wt{ib}q{q}",
                                 name=f"wt{ib}q{q}") for q in range(2)]
                   for ib in range(KB)]

            IC = 1024              # i-chunk width for the elementwise pass
            NIC = IN_F // IC       # 4 chunks
            tr_idx = 0
            for ob in range(OS // 128):          # 8 o-row blocks
                q, off = divmod(ob * 128, 512)
                # load the 4 rho chunks, then batch Exp x4 and Ln x4 so the
                # ACT table set switches twice per ob instead of per chunk
                rho_cs = []
                for ic in range(NIC):
                    rho_c = prep_rho.tile([128, IC], FP32, tag=f"rho{ic}",
                                          name=f"rho_{ob}_{ic}")
                    nc.sync.dma_start(
                        rho_c[:], rho[ob * 128:(ob + 1) * 128,
                                      ic * IC:(ic + 1) * IC])
                    rho_cs.append(rho_c)
                for rho_c in rho_cs:
                    nc.scalar.activation(rho_c[:], rho_c[:], AF.Exp)
                for rho_c in rho_cs:
                    nc.scalar.activation(rho_c[:], rho_c[:], AF.Ln, bias=1.0)
                for ic in range(NIC):
                    mu_c = prep_in.tile([128, IC], FP32, tag="mu")
                    eps_c = prep_in.tile([128, IC], FP32, tag="eps")
                    sl = (slice(ob * 128, (ob + 1) * 128),
                          slice(ic * IC, (ic + 1) * IC))
                    nc.sync.dma_start(mu_c[:], mu[sl])
                    nc.sync.dma_start(eps_c[:], eps[sl])
                    nc.vector.tensor_mul(eps_c[:], rho_cs[ic][:], eps_c[:])
                    wf = prep_w.tile([128, IC], F16, tag="wf")
                    nc.vector.tensor_add(wf[:], eps_c[:], mu_c[:])
                    # transpose each [128 o, 128 i] block into WT[ib][q]
                    for j in range(IC // 128):
                        ib = ic * (IC // 128) + j
                        rings[tr_idx % 2].dma_start(
                            wts[ib][q][:, off:off + 128],
                            wf[:, j * 128:(j + 1) * 128],
                            transpose=True,
                        )
                        tr_idx += 1

            # ---- main loop: super-tiles of NB rows of x
            n_super = NS // NB
            subs = NB // 128
            for s in range(n_super):
                xt = xt_pool.tile([128, KB * NB], F16)
                for ib in range(KB):
                    rings[ib % 2].dma_start(
                        xt[:, ib * NB:(ib + 1) * NB],
                        xb[s * NB:(s + 1) * NB, ib * 128:(ib + 1) * 128],
                        transpose=True,
                    )
                for sub in range(subs):
                    ps0 = psum_pool.tile([128, 512], FP32)
                    ps1 = psum_pool.tile([128, 512], FP32)
                    for ib in range(KB):
                        xs = xt[:, ib * NB + sub * 128: ib * NB + (sub + 1) * 128]
                        nc.tensor.matmul(ps0[:], xs, wts[ib][0][:],
                                         start=(ib == 0), stop=(ib == KB - 1))
                        nc.tensor.matmul(ps1[:], xs, wts[ib][1][:],
                                         start=(ib == 0), stop=(ib == KB - 1))
                    ob_t = out_pool.tile([128, OS], FP32)
                    nc.vector.tensor_add(ob_t[:, 0:512], ps0[:], bias_t[:, 0:512])
                    nc.vector.tensor_add(ob_t[:, 512:1024], ps1[:], bias_t[:, 512:1024])
                    row = (s * subs + sub) * 128
                    rings[sub % 2].dma_start(out[row:row + 128, :], ob_t[:])

    nc.compile()
    return nc


_NC = None


def _get_nc():
    global _NC
    if _NC is None:
        _NC = _build_nc()
    return _NC


def kernel(x, weight_mu, weight_rho, bias_mu, bias_rho, eps_w, eps_b,
           _trace=False, _trace_kwargs=None):
    x = np.asarray(x, dtype=np.float32)
    weight_mu = np.asarray(weight_mu, dtype=np.float32)
    weight_rho = np.asarray(weight_rho, dtype=np.float32)
    bias_mu = np.asarray(bias_mu, dtype=np.float32)
    bias_rho = np.asarray(bias_rho, dtype=np.float32)
    eps_w = np.asarray(eps_w, dtype=np.float32)
    eps_b = np.asarray(eps_b, dtype=np.float32)

    nc = _get_nc()
    xb = x.astype(np.float16)

    in_maps = []
    for c in range(N_CORES):
        r, q = divmod(c, C)
        osl = slice(q * OS, (q + 1) * OS)
        in_maps.append({
            "xb": xb[r * NS:(r + 1) * NS],
            "mu": weight_mu[osl],
            "rho": weight_rho[osl],
            "eps": eps_w[osl],
            "bmu": np.ascontiguousarray(np.broadcast_to(bias_mu[osl], (128, OS))),
            "brho": np.ascontiguousarray(np.broadcast_to(bias_rho[osl], (128, OS))),
            "beps": np.ascontiguousarray(np.broadcast_to(eps_b[osl], (128, OS))),
        })

    kwargs = {}
    if _trace:
        kwargs["trace"] = True
        if _trace_kwargs:
            kwargs.update(_trace_kwargs)
    res = bass_utils.run_bass_kernel_spmd(
        nc, in_maps, core_ids=list(range(N_CORES)), **kwargs)

    out = np.empty((N, OUT_F), np.float32)
    for c in range(N_CORES):
        r, q = divmod(c, C)
        out[r * NS:(r + 1) * NS, q * OS:(q + 1) * OS] = res.results[c]["out"]
    if _trace:
        return out, res
    return out


# revision 7
# speedup vs baseline: 1.6738x; 1.2124x over previous
"""Bayesian linear layer on 8 TRN2 NeuronCores.

Computes  out = x @ (mu + softplus(rho) * eps_w).T + (bmu + softplus(brho) * eps_b)
for x [16384, 4096], weights [4096, 4096].

Sharding: 2-way split of the batch dim (N) x 4-way split of out_features.
Each core computes an [8192, 1024] fp32 output shard:
  - weight shard W^T is generated on-device: softplus via Exp + Ln(x+1) on the
    ACT engine (table sets batched), FMA on DVE with fp16 output, staged
    through a DRAM scratch, then xbar transpose-loaded as 64 resident tiles
    [128 (in_f), 512 (out_f)] fp16.
  - x is shipped fp16 and xbar transpose-loaded straight from DRAM in
    [1024 x 128] panels (4 k-quarter tiles per 1024-row super-tile).
  - matmuls are fp16, N=512, fp32 PSUM accumulation over 32 k-blocks; the two
    output halves (q=0/1) run as separate phases over 8 PSUM banks so the
    first phase only needs half the prepped weights; bias is added during the
    PSUM->SBUF copy on DVE.
All DMAs stay on the SP HWDGE ring: splitting across the SP+ACT rings
corrupts results on this stack (completion tracking assumes one ring).
"""

import numpy as np

import concourse.bacc as bacc
import concourse.tile as tile
from concourse import mybir
from concourse import bass_utils

R, C = 2, 4                      # grid: R-way split of N, C-way split of out_f
N, IN_F, OUT_F = 16384, 4096, 4096
NS, OS = N // R, OUT_F // C      # per-core shards: 8192 rows, 1024 out cols
KB = IN_F // 128                 # 32 k-blocks
NB = 1024                       # rows per super-tile
NKQ = 4                          # k-quarters per super-tile
KQ = KB // NKQ                   # 8 k-blocks per quarter
N_CORES = 8

FP32 = mybir.dt.float32
F16 = mybir.dt.float16


def _build_nc():
    nc = bacc.Bacc("TRN2", target_bir_lowering=False, debug=False)

    xb = nc.dram_tensor("xb", [NS, IN_F], F16, kind="ExternalInput").ap()
    mu = nc.dram_tensor("mu", [OS, IN_F], FP32, kind="ExternalInput").ap()
    rho = nc.dram_tensor("rho", [OS, IN_F], FP32, kind="ExternalInput").ap()
    eps = nc.dram_tensor("eps", [OS, IN_F], FP32, kind="ExternalInput").ap()
    bmu = nc.dram_tensor("bmu", [128, OS], FP32, kind="ExternalInput").ap()
    brho = nc.dram_tensor("brho", [128, OS], FP32, kind="ExternalInput").ap()
    beps = nc.dram_tensor("beps", [128, OS], FP32, kind="ExternalInput").ap()
    out = nc.dram_tensor("out", [NS, OS], FP32, kind="ExternalOutput").ap()

    AF = mybir.ActivationFunctionType
    n_super = NS // NB
    subs = NB // 128

    with tile.TileContext(nc) as tc:
        with (
            tc.tile_pool(name="wt", bufs=1) as wt_pool,
            tc.tile_pool(name="bias", bufs=1) as bias_pool,
            tc.tile_pool(name="prep_rho", bufs=1) as prep_rho,
            tc.tile_pool(name="prep_in", bufs=2) as prep_in,
            tc.tile_pool(name="prep_w", bufs=2) as prep_w,
            tc.tile_pool(name="w16", bufs=1, space="DRAM") as w16_pool,
            tc.tile_pool(name="xt", bufs=1) as xt_pool,
            tc.tile_pool(name="outp", bufs=4) as out_pool,
            tc.tile_pool(name="psum", bufs=1, space="PSUM") as psum_pool,
        ):
            # ---- bias: b = bmu + softplus(brho) * beps, replicated [128, OS]
            bmu_t = bias_pool.tile([128, OS], FP32, tag="bmu")
            brho_t = bias_pool.tile([128, OS], FP32, tag="brho")
            beps_t = bias_pool.tile([128, OS], FP32, tag="beps")
            nc.sync.dma_start(bmu_t[:], bmu[:])
            nc.sync.dma_start(brho_t[:], brho[:])
            nc.sync.dma_start(beps_t[:], beps[:])
            nc.scalar.activation(brho_t[:], brho_t[:], AF.Exp)
            nc.scalar.activation(brho_t[:], brho_t[:], AF.Ln, bias=1.0)
            nc.vector.tensor_mul(beps_t[:], brho_t[:], beps_t[:])
            bias_t = bias_pool.tile([128, OS], FP32, tag="bias")
            nc.vector.tensor_add(bias_t[:], beps_t[:], bmu_t[:])

            # ---- W^T: computed in [o, i] layout, staged to DRAM as fp16,
            # then transpose-loaded into 64 resident [128, 512] tiles [ib][q].
            wts = [[wt_pool.tile([128, 512], F16, tag=f"wt{ib}q{q}",
                                 name=f"wt{ib}q{q}") for q in range(2)]
                   for ib in range(KB)]
            w16q = [w16_pool.tile([512, IN_F], F16, tag=f"w16q{q}",
                                  name=f"w16q{q}") for q in range(2)]

            IC = 1024
            NIC = IN_F // IC

            def prep_ob(ob):
                # softplus(rho) in place (batched Exp then Ln per ob), then
                # w = mu + sp * eps -> fp16, stored to the DRAM scratch.
                q, roff = divmod(ob * 128, 512)
                rho_cs = []
                for ic in range(NIC):
                    rho_c = prep_rho.tile([128, IC], FP32, tag=f"rho{ic}",
                                          name=f"rho_{ob}_{ic}")
                    nc.sync.dma_start(
                        rho_c[:], rho[ob * 128:(ob + 1) * 128,
                                      ic * IC:(ic + 1) * IC])
                    rho_cs.append(rho_c)
                for rho_c in rho_cs:
                    nc.scalar.activation(rho_c[:], rho_c[:], AF.Exp)
                for rho_c in rho_cs:
                    nc.scalar.activation(rho_c[:], rho_c[:], AF.Ln, bias=1.0)
                for ic in range(NIC):
                    mu_c = prep_in.tile([128, IC], FP32, tag="mu")
                    eps_c = prep_in.tile([128, IC], FP32, tag="eps")
                    sl = (slice(ob * 128, (ob + 1) * 128),
                          slice(ic * IC, (ic + 1) * IC))
                    nc.sync.dma_start(mu_c[:], mu[sl])
                    nc.sync.dma_start(eps_c[:], eps[sl])
                    nc.vector.tensor_mul(eps_c[:], rho_cs[ic][:], eps_c[:])
                    wf = prep_w.tile([128, IC], F16, tag="wf")
                    nc.vector.tensor_add(wf[:], eps_c[:], mu_c[:])
                    nc.sync.dma_start(
                        w16q[q][roff:roff + 128, ic * IC:(ic + 1) * IC], wf[:])

            def load_wt_q(q):
                for ib in range(KB):
                    nc.sync.dma_start(wts[ib][q][:],
                                      w16q[q][:, ib * 128:(ib + 1) * 128],
                                      transpose=True)

            def xt_panels(s):
                xtq = []
                for kq in range(NKQ):
                    xtt = xt_pool.tile([128, KQ * NB], F16, tag=f"kq{kq}",
                                       name=f"xt_s{s}_k{kq}")
                    for j in range(KQ):
                        ib = kq * KQ + j
                        nc.sync.dma_start(
                            xtt[:, j * NB:(j + 1) * NB],
                            xb[s * NB:(s + 1) * NB, ib * 128:(ib + 1) * 128],
                            transpose=True)
                    xtq.append(xtt)
                return xtq

            # emission order: q0 weights -> first super-tile's x panels ->
            # q1 weights, so the ring produces matmul inputs early.
            for ob in range(4):
                prep_ob(ob)
            load_wt_q(0)
            xtq0 = xt_panels(0)
            for ob in range(4, 8):
                prep_ob(ob)
            load_wt_q(1)

            # ---- main loop
            for s in range(n_super):
                xtq = xtq0 if s == 0 else xt_panels(s)
                for q in range(2):
                    psq = [psum_pool.tile([128, 512], FP32, tag=f"ps{sub}",
                                          name=f"ps_{s}_{q}_{sub}")
                           for sub in range(subs)]
                    for kq in range(NKQ):
                        for sub in range(subs):
                            for j in range(KQ):
                                ib = kq * KQ + j
                                xs = xtq[kq][:, j * NB + sub * 128:
                                             j * NB + (sub + 1) * 128]
                                nc.tensor.matmul(psq[sub][:], xs, wts[ib][q][:],
                                                 start=(ib == 0),
                                                 stop=(ib == KB - 1))
                    for sub in range(subs):
                        ot = out_pool.tile([128, 512], FP32, tag="ot",
                                           name=f"ot_{s}_{q}_{sub}")
                        nc.vector.tensor_add(
                            ot[:], psq[sub][:], bias_t[:, q * 512:(q + 1) * 512])
                        row = (s * subs + sub) * 128
                        nc.sync.dma_start(
                            out[row:row + 128, q * 512:(q + 1) * 512], ot[:])

    nc.compile()
    return nc


_NC = None


def _get_nc():
    global _NC
    if _NC is None:
        _NC = _build_nc()
    return _NC


def kernel(x, weight_mu, weight_rho, bias_mu, bias_rho, eps_w, eps_b,
           _trace=False, _trace_kwargs=None):
    x = np.asarray(x, dtype=np.float32)
    weight_mu = np.asarray(weight_mu, dtype=np.float32)
    weight_rho = np.asarray(weight_rho, dtype=np.float32)
    bias_mu = np.asarray(bias_mu, dtype=np.float32)
    bias_rho = np.asarray(bias_rho, dtype=np.float32)
    eps_w = np.asarray(eps_w, dtype=np.float32)
    eps_b = np.asarray(eps_b, dtype=np.float32)

    nc = _get_nc()
    xb = x.astype(np.float16)

    in_maps = []
    for c in range(N_CORES):
        r, q = divmod(c, C)
        osl = slice(q * OS, (q + 1) * OS)
        in_maps.append({
            "xb": xb[r * NS:(r + 1) * NS],
            "mu": weight_mu[osl],
            "rho": weight_rho[osl],
            "eps": eps_w[osl],
            "bmu": np.ascontiguousarray(np.broadcast_to(bias_mu[osl], (128, OS))),
            "brho": np.ascontiguousarray(np.broadcast_to(bias_rho[osl], (128, OS))),
            "beps": np.ascontiguousarray(np.broadcast_to(eps_b[osl], (128, OS))),
        })

    kwargs = {}
    if _trace:
        kwargs["trace"] = True
        if _trace_kwargs:
            kwargs.update(_trace_kwargs)
    res = bass_utils.run_bass_kernel_spmd(
        nc, in_maps, core_ids=list(range(N_CORES)), **kwargs)

    out = np.empty((N, OUT_F), np.float32)
    for c in range(N_CORES):
        r, q = divmod(c, C)
        out[r * NS:(r + 1) * NS, q * OS:(q + 1) * OS] = res.results[c]["out"]
    if _trace:
        return out, res
    return out


# revision 8
# speedup vs baseline: 1.7619x; 1.0526x over previous
"""Bayesian linear layer on 8 TRN2 NeuronCores.

Computes  out = x @ (mu + softplus(rho) * eps_w).T + (bmu + softplus(brho) * eps_b)
for x [16384, 4096], weights [4096, 4096].

Sharding: 2-way split of the batch dim (N) x 4-way split of out_features.
Each core computes an [8192, 1024] fp32 output shard:
  - weight shard W^T is generated on-device: softplus via Exp + Ln(x+1) on the
    ACT engine (table sets batched), FMA on DVE with fp16 output, staged
    through a DRAM scratch, then xbar transpose-loaded as 64 resident tiles
    [128 (in_f), 512 (out_f)] fp16.
  - x is shipped fp16 and xbar transpose-loaded straight from DRAM in
    [1024 x 128] panels (4 k-quarter tiles per 1024-row super-tile).
  - matmuls are fp16, N=512, fp32 PSUM accumulation over 32 k-blocks; the two
    output halves (q=0/1) run as separate phases over 8 PSUM banks so the
    first phase only needs half the prepped weights; bias is added during the
    PSUM->SBUF copy on DVE.
All DMAs stay on the SP HWDGE ring: splitting across the SP+ACT rings
corrupts results on this stack (completion tracking assumes one ring).
"""

import numpy as np

import concourse.bacc as bacc
import concourse.tile as tile
from concourse import mybir
from concourse import bass_utils

R, C = 2, 4                      # grid: R-way split of N, C-way split of out_f
N, IN_F, OUT_F = 16384, 4096, 4096
NS, OS = N // R, OUT_F // C      # per-core shards: 8192 rows, 1024 out cols
KB = IN_F // 128                 # 32 k-blocks
NB = 1024                       # rows per super-tile
NKQ = 4                          # k-quarters per super-tile
KQ = KB // NKQ                   # 8 k-blocks per quarter
N_CORES = 8

FP32 = mybir.dt.float32
F16 = mybir.dt.float16


def _build_nc():
    nc = bacc.Bacc("TRN2", target_bir_lowering=False, debug=False)

    xb = nc.dram_tensor("xb", [NS, IN_F], F16, kind="ExternalInput").ap()
    mu = nc.dram_tensor("mu", [OS, IN_F], FP32, kind="ExternalInput").ap()
    rho = nc.dram_tensor("rho", [OS, IN_F], FP32, kind="ExternalInput").ap()
    eps = nc.dram_tensor("eps", [OS, IN_F], FP32, kind="ExternalInput").ap()
    bmu = nc.dram_tensor("bmu", [128, OS], FP32, kind="ExternalInput").ap()
    brho = nc.dram_tensor("brho", [128, OS], FP32, kind="ExternalInput").ap()
    beps = nc.dram_tensor("beps", [128, OS], FP32, kind="ExternalInput").ap()
    out = nc.dram_tensor("out", [NS, OS], FP32, kind="ExternalOutput").ap()

    AF = mybir.ActivationFunctionType
    n_super = NS // NB
    subs = NB // 128

    with tile.TileContext(nc) as tc:
        with (
            tc.tile_pool(name="wt", bufs=1) as wt_pool,
            tc.tile_pool(name="bias", bufs=1) as bias_pool,
            tc.tile_pool(name="prep_rho", bufs=1) as prep_rho,
            tc.tile_pool(name="prep_in", bufs=2) as prep_in,
            tc.tile_pool(name="prep_w", bufs=2) as prep_w,
            tc.tile_pool(name="w16", bufs=1, space="DRAM") as w16_pool,
            tc.tile_pool(name="xt", bufs=1) as xt_pool,
            tc.tile_pool(name="outp", bufs=4) as out_pool,
            tc.tile_pool(name="psum", bufs=1, space="PSUM") as psum_pool,
        ):
            # ---- bias: b = bmu + softplus(brho) * beps, replicated [128, OS]
            bmu_t = bias_pool.tile([128, OS], FP32, tag="bmu")
            brho_t = bias_pool.tile([128, OS], FP32, tag="brho")
            beps_t = bias_pool.tile([128, OS], FP32, tag="beps")
            nc.sync.dma_start(bmu_t[:], bmu[:])
            nc.sync.dma_start(brho_t[:], brho[:])
            nc.sync.dma_start(beps_t[:], beps[:])
            nc.scalar.activation(brho_t[:], brho_t[:], AF.Exp)
            nc.scalar.activation(brho_t[:], brho_t[:], AF.Ln, bias=1.0)
            nc.vector.tensor_mul(beps_t[:], brho_t[:], beps_t[:])
            bias_t = bias_pool.tile([128, OS], FP32, tag="bias")
            nc.vector.tensor_add(bias_t[:], beps_t[:], bmu_t[:])

            # ---- W^T: computed in [o, i] layout, staged to DRAM as fp16,
            # then transpose-loaded into 64 resident [128, 512] tiles [ib][q].
            wts = [[wt_pool.tile([128, 512], F16, tag=f"wt{ib}q{q}",
                                 name=f"wt{ib}q{q}") for q in range(2)]
                   for ib in range(KB)]
            w16q = [w16_pool.tile([512, IN_F], F16, tag=f"w16q{q}",
                                  name=f"w16q{q}") for q in range(2)]

            IC = 1024
            NIC = IN_F // IC

            def prep_ob(ob):
                # softplus(rho) in place (batched Exp then Ln per ob), then
                # w = mu + sp * eps -> fp16, stored to the DRAM scratch.
                q, roff = divmod(ob * 128, 512)
                rho_cs = []
                for ic in range(NIC):
                    rho_c = prep_rho.tile([128, IC], FP32, tag=f"rho{ic}",
                                          name=f"rho_{ob}_{ic}")
                    nc.sync.dma_start(
                        rho_c[:], rho[ob * 128:(ob + 1) * 128,
                                      ic * IC:(ic + 1) * IC])
                    rho_cs.append(rho_c)
                for rho_c in rho_cs:
                    nc.scalar.activation(rho_c[:], rho_c[:], AF.Exp)
                for rho_c in rho_cs:
                    nc.scalar.activation(rho_c[:], rho_c[:], AF.Ln, bias=1.0)
                for ic in range(NIC):
                    mu_c = prep_in.tile([128, IC], FP32, tag="mu")
                    eps_c = prep_in.tile([128, IC], FP32, tag="eps")
                    sl = (slice(ob * 128, (ob + 1) * 128),
                          slice(ic * IC, (ic + 1) * IC))
                    nc.sync.dma_start(mu_c[:], mu[sl])
                    nc.sync.dma_start(eps_c[:], eps[sl])
                    nc.vector.tensor_mul(eps_c[:], rho_cs[ic][:], eps_c[:])
                    wf = prep_w.tile([128, IC], F16, tag="wf")
                    nc.vector.tensor_add(wf[:], eps_c[:], mu_c[:])
                    nc.sync.dma_start(
                        w16q[q][roff:roff + 128, ic * IC:(ic + 1) * IC], wf[:])

            def load_wt_q(q):
                for ib in range(KB):
                    nc.sync.dma_start(wts[ib][q][:],
                                      w16q[q][:, ib * 128:(ib + 1) * 128],
                                      transpose=True)

            def xt_panel(s, kq):
                xtt = xt_pool.tile([128, KQ * NB], F16, tag=f"kq{kq}",
                                   name=f"xt_s{s}_k{kq}")
                for j in range(KQ):
                    ib = kq * KQ + j
                    nc.sync.dma_start(
                        xtt[:, j * NB:(j + 1) * NB],
                        xb[s * NB:(s + 1) * NB, ib * 128:(ib + 1) * 128],
                        transpose=True)
                return xtt

            def xt_panels(s):
                return [xt_panel(s, kq) for kq in range(NKQ)]

            # emission order: q0 weights first, then alternate the q1 weight
            # prep with the first super-tile's x panels so the ring feeds the
            # PE from ~100us while the rest of the weights stream in behind.
            for ob in range(4):
                prep_ob(ob)
            load_wt_q(0)
            xtq0 = []
            for kq in range(NKQ):
                prep_ob(4 + kq)
                xtq0.append(xt_panel(0, kq))
            load_wt_q(1)

            # ---- main loop
            for s in range(n_super):
                xtq = xtq0 if s == 0 else xt_panels(s)
                for q in range(2):
                    psq = [psum_pool.tile([128, 512], FP32, tag=f"ps{sub}",
                                          name=f"ps_{s}_{q}_{sub}")
                           for sub in range(subs)]
                    for kq in range(NKQ):
                        for sub in range(subs):
                            for j in range(KQ):
                                ib = kq * KQ + j
                                xs = xtq[kq][:, j * NB + sub * 128:
                                             j * NB + (sub + 1) * 128]
                                nc.tensor.matmul(psq[sub][:], xs, wts[ib][q][:],
                                                 start=(ib == 0),
                                                 stop=(ib == KB - 1))
                    for sub in range(subs):
                        ot = out_pool.tile([128, 512], FP32, tag="ot",
                                           name=f"ot_{s}_{q}_{sub}")
                        nc.vector.tensor_add(
                            ot[:], psq[sub][:], bias_t[:, q * 512:(q + 1) * 512])
                        row = (s * subs + sub) * 128
                        nc.sync.dma_start(
                            out[row:row + 128, q * 512:(q + 1) * 512], ot[:])

    nc.compile()
    return nc


_NC = None


def _get_nc():
    global _NC
    if _NC is None:
        _NC = _build_nc()
    return _NC


def kernel(x, weight_mu, weight_rho, bias_mu, bias_rho, eps_w, eps_b,
           _trace=False, _trace_kwargs=None):
    x = np.asarray(x, dtype=np.float32)
    weight_mu = np.asarray(weight_mu, dtype=np.float32)
    weight_rho = np.asarray(weight_rho, dtype=np.float32)
    bias_mu = np.asarray(bias_mu, dtype=np.float32)
    bias_rho = np.asarray(bias_rho, dtype=np.float32)
    eps_w = np.asarray(eps_w, dtype=np.float32)
    eps_b = np.asarray(eps_b, dtype=np.float32)

    nc = _get_nc()
    xb = x.astype(np.float16)

    in_maps = []
    for c in range(N_CORES):
        r, q = divmod(c, C)
        osl = slice(q * OS, (q + 1) * OS)
        in_maps.append({
            "xb": xb[r * NS:(r + 1) * NS],
            "mu": weight_mu[osl],
            "rho": weight_rho[osl],
            "eps": eps_w[osl],
            "bmu": np.ascontiguousarray(np.broadcast_to(bias_mu[osl], (128, OS))),
            "brho": np.ascontiguousarray(np.broadcast_to(bias_rho[osl], (128, OS))),
            "beps": np.ascontiguousarray(np.broadcast_to(eps_b[osl], (128, OS))),
        })

    kwargs = {}
    if _trace:
        kwargs["trace"] = True
        if _trace_kwargs:
            kwargs.update(_trace_kwargs)
    res = bass_utils.run_bass_kernel_spmd(
        nc, in_maps, core_ids=list(range(N_CORES)), **kwargs)

    out = np.empty((N, OUT_F), np.float32)
    for c in range(N_CORES):
        r, q = divmod(c, C)
        out[r * NS:(r + 1) * NS, q * OS:(q + 1) * OS] = res.results[c]["out"]
    if _trace:
        return out, res
    return out


# revision 9
# speedup vs baseline: 1.8401x; 1.0444x over previous
"""Bayesian linear layer on 8 TRN2 NeuronCores.

Computes  out = x @ (mu + softplus(rho) * eps_w).T + (bmu + softplus(brho) * eps_b)
for x [16384, 4096], weights [4096, 4096].

Sharding: 2-way split of the batch dim (N) x 4-way split of out_features.
Each core computes an [8192, 1024] fp32 output shard:
  - weight shard W^T is generated on-device: softplus via Exp + Ln(x+1) on the
    ACT engine (table sets batched), FMA on DVE with fp16 output, staged
    through a DRAM scratch, then xbar transpose-loaded as 64 resident tiles
    [128 (in_f), 512 (out_f)] fp16.
  - x is shipped fp16 and xbar transpose-loaded straight from DRAM in
    [1024 x 128] panels (4 k-quarter tiles per 1024-row super-tile).
  - matmuls are fp16, N=512, fp32 PSUM accumulation over 32 k-blocks; the two
    output halves (q=0/1) run as separate phases over 8 PSUM banks so the
    first phase only needs half the prepped weights; bias is added during the
    PSUM->SBUF copy on DVE.
All DMAs stay on the SP HWDGE ring: splitting across the SP+ACT rings
corrupts results on this stack (completion tracking assumes one ring).
"""

import numpy as np

import concourse.bacc as bacc
import concourse.tile as tile
from concourse import mybir
from concourse import bass_utils

R, C = 2, 4                      # grid: R-way split of N, C-way split of out_f
N, IN_F, OUT_F = 16384, 4096, 4096
NS, OS = N // R, OUT_F // C      # per-core shards: 8192 rows, 1024 out cols
KB = IN_F // 128                 # 32 k-blocks
NB = 1024                       # rows per super-tile
NKQ = 4                          # k-quarters per super-tile
KQ = KB // NKQ                   # 8 k-blocks per quarter
N_CORES = 8

FP32 = mybir.dt.float32
F16 = mybir.dt.float16


def _build_nc():
    nc = bacc.Bacc("TRN2", target_bir_lowering=False, debug=False)

    xb = nc.dram_tensor("xb", [NS, IN_F], F16, kind="ExternalInput").ap()
    mu = nc.dram_tensor("mu", [OS, IN_F], FP32, kind="ExternalInput").ap()
    rho = nc.dram_tensor("rho", [OS, IN_F], FP32, kind="ExternalInput").ap()
    eps = nc.dram_tensor("eps", [OS, IN_F], FP32, kind="ExternalInput").ap()
    bmu = nc.dram_tensor("bmu", [128, OS], FP32, kind="ExternalInput").ap()
    brho = nc.dram_tensor("brho", [128, OS], FP32, kind="ExternalInput").ap()
    beps = nc.dram_tensor("beps", [128, OS], FP32, kind="ExternalInput").ap()
    out = nc.dram_tensor("out", [NS, OS], FP32, kind="ExternalOutput").ap()

    AF = mybir.ActivationFunctionType
    n_super = NS // NB
    subs = NB // 128

    with tile.TileContext(nc) as tc:
        with (
            tc.tile_pool(name="wt", bufs=1) as wt_pool,
            tc.tile_pool(name="bias", bufs=1) as bias_pool,
            tc.tile_pool(name="prep_rho", bufs=1) as prep_rho,
            tc.tile_pool(name="prep_in", bufs=2) as prep_in,
            tc.tile_pool(name="prep_w", bufs=2) as prep_w,
            tc.tile_pool(name="w16", bufs=1, space="DRAM") as w16_pool,
            tc.tile_pool(name="xt", bufs=1) as xt_pool,
            tc.tile_pool(name="outp", bufs=4) as out_pool,
            tc.tile_pool(name="psum", bufs=1, space="PSUM") as psum_pool,
        ):
            # ---- bias: b = bmu + softplus(brho) * beps, replicated [128, OS]
            bmu_t = bias_pool.tile([128, OS], FP32, tag="bmu")
            brho_t = bias_pool.tile([128, OS], FP32, tag="brho")
            beps_t = bias_pool.tile([128, OS], FP32, tag="beps")
            nc.sync.dma_start(bmu_t[:], bmu[:])
            nc.sync.dma_start(brho_t[:], brho[:])
            nc.sync.dma_start(beps_t[:], beps[:])
            nc.scalar.activation(brho_t[:], brho_t[:], AF.Exp)
            nc.scalar.activation(brho_t[:], brho_t[:], AF.Ln, bias=1.0)
            nc.vector.tensor_mul(beps_t[:], brho_t[:], beps_t[:])
            bias_t = bias_pool.tile([128, OS], FP32, tag="bias")
            nc.vector.tensor_add(bias_t[:], beps_t[:], bmu_t[:])

            # ---- W^T: computed in [o, i] layout, staged to DRAM as fp16,
            # then transpose-loaded into 32 resident [128, 1024] tiles.
            wts = [wt_pool.tile([128, OS], F16, tag=f"wt{ib}",
                                name=f"wt{ib}") for ib in range(KB)]
            w16 = w16_pool.tile([OS, IN_F], F16, tag="w16", name="w16")

            IC = 1024
            NIC = IN_F // IC

            def prep_ob(ob):
                # softplus(rho) in place (batched Exp then Ln per ob), then
                # w = mu + sp * eps -> fp16, stored to the DRAM scratch.
                rho_cs = []
                for ic in range(NIC):
                    rho_c = prep_rho.tile([128, IC], FP32, tag=f"rho{ic}",
                                          name=f"rho_{ob}_{ic}")
                    nc.sync.dma_start(
                        rho_c[:], rho[ob * 128:(ob + 1) * 128,
                                      ic * IC:(ic + 1) * IC])
                    rho_cs.append(rho_c)
                for rho_c in rho_cs:
                    nc.scalar.activation(rho_c[:], rho_c[:], AF.Exp)
                for rho_c in rho_cs:
                    nc.scalar.activation(rho_c[:], rho_c[:], AF.Ln, bias=1.0)
                for ic in range(NIC):
                    mu_c = prep_in.tile([128, IC], FP32, tag="mu")
                    eps_c = prep_in.tile([128, IC], FP32, tag="eps")
                    sl = (slice(ob * 128, (ob + 1) * 128),
                          slice(ic * IC, (ic + 1) * IC))
                    nc.sync.dma_start(mu_c[:], mu[sl])
                    nc.sync.dma_start(eps_c[:], eps[sl])
                    nc.vector.tensor_mul(eps_c[:], rho_cs[ic][:], eps_c[:])
                    wf = prep_w.tile([128, IC], F16, tag="wf")
                    nc.vector.tensor_add(wf[:], eps_c[:], mu_c[:])
                    nc.sync.dma_start(
                        w16[ob * 128:(ob + 1) * 128, ic * IC:(ic + 1) * IC],
                        wf[:])

            def xt_panel(s, kq):
                xtt = xt_pool.tile([128, KQ * NB], F16, tag=f"kq{kq}",
                                   name=f"xt_s{s}_k{kq}")
                for j in range(KQ):
                    ib = kq * KQ + j
                    nc.sync.dma_start(
                        xtt[:, j * NB:(j + 1) * NB],
                        xb[s * NB:(s + 1) * NB, ib * 128:(ib + 1) * 128],
                        transpose=True)
                return xtt

            def xt_panels(s):
                return [xt_panel(s, kq) for kq in range(NKQ)]

            # emission order: all weight prep, the 32 big weight transpose
            # loads, then the first super-tile's x panels; the ring clears
            # this in ~170us and stays ahead of the PE afterwards.
            for ob in range(OS // 128):
                prep_ob(ob)
            for ib in range(KB):
                nc.sync.dma_start(wts[ib][:],
                                  w16[:, ib * 128:(ib + 1) * 128],
                                  transpose=True)
            xtq0 = xt_panels(0)

            # ---- main loop
            for s in range(n_super):
                xtq = xtq0 if s == 0 else xt_panels(s)
                for q in range(2):
                    psq = [psum_pool.tile([128, 512], FP32, tag=f"ps{sub}",
                                          name=f"ps_{s}_{q}_{sub}")
                           for sub in range(subs)]
                    for kq in range(NKQ):
                        for sub in range(subs):
                            for j in range(KQ):
                                ib = kq * KQ + j
                                xs = xtq[kq][:, j * NB + sub * 128:
                                             j * NB + (sub + 1) * 128]
                                nc.tensor.matmul(
                                    psq[sub][:], xs,
                                    wts[ib][:, q * 512:(q + 1) * 512],
                                    start=(ib == 0), stop=(ib == KB - 1))
                    for sub in range(subs):
                        ot = out_pool.tile([128, 512], FP32, tag="ot",
                                           name=f"ot_{s}_{q}_{sub}")
                        nc.vector.tensor_add(
                            ot[:], psq[sub][:], bias_t[:, q * 512:(q + 1) * 512])
                        row = (s * subs + sub) * 128
                        nc.sync.dma_start(
                            out[row:row + 128, q * 512:(q + 1) * 512], ot[:])

    nc.compile()
    return nc


_NC = None


def _get_nc():
    global _NC
    if _NC is None:
        _NC = _build_nc()
    return _NC


def kernel(x, weight_mu, weight_rho, bias_mu, bias_rho, eps_w, eps_b,
           _trace=False, _trace_kwargs=None):
    x = np.asarray(x, dtype=np.float32)
    weight_mu = np.asarray(weight_mu, dtype=np.float32)
    weight_rho = np.asarray(weight_rho, dtype=np.float32)
    bias_mu = np.asarray(bias_mu, dtype=np.float32)
    bias_rho = np.asarray(bias_rho, dtype=np.float32)
    eps_w = np.asarray(eps_w, dtype=np.float32)
    eps_b = np.asarray(eps_b, dtype=np.float32)

    nc = _get_nc()
    xb = x.astype(np.float16)

    in_maps = []
    for c in range(N_CORES):
        r, q = divmod(c, C)
        osl = slice(q * OS, (q + 1) * OS)
        in_maps.append({
            "xb": xb[r * NS:(r + 1) * NS],
            "mu": weight_mu[osl],
            "rho": weight_rho[osl],
            "eps": eps_w[osl],
            "bmu": np.ascontiguousarray(np.broadcast_to(bias_mu[osl], (128, OS))),
            "brho": np.ascontiguousarray(np.broadcast_to(bias_rho[osl], (128, OS))),
            "beps": np.ascontiguousarray(np.broadcast_to(eps_b[osl], (128, OS))),
        })

    kwargs = {}
    if _trace:
        kwargs["trace"] = True
        if _trace_kwargs:
            kwargs.update(_trace_kwargs)
    res = bass_utils.run_bass_kernel_spmd(
        nc, in_maps, core_ids=list(range(N_CORES)), **kwargs)

    out = np.empty((N, OUT_F), np.float32)
    for c in range(N_CORES):
        r, q = divmod(c, C)
        out[r * NS:(r + 1) * NS, q * OS:(q + 1) * OS] = res.results[c]["out"]
    if _trace:
        return out, res
    return out
